# revision 30
# baseline (speedup 1.0000x reference)
"""DeepSeek-style MLA transformer block on 8 Trainium2 NeuronCores.

Strategy (feature-major activations on device; weights host-pre-transposed so
every matmul chains without on-device transposes):

  Stage A (token-sharded, 256 tok/core): attn_norm -> wq_a -> q_norm and
    wkv_a -> kv_norm + rope(k_pe), all matmuls fp16 with k-slab weight DMAs
    (full-bandwidth lines) accumulating into per-m-chunk PSUM banks; the q
    and kv latents AllGather in fp16 (q first so stage B starts sooner).
  Stage B (head-sharded, 2 heads/core): wq_b (+rope q) in fp16, expand
    k_nope/v from the gathered kv latent (fp16 matmuls), exact-causal
    attention (skip fully-masked 256-col chunks; mask DMA only for the
    diagonal chunk of each query row-block, shared across heads/batch),
    fp32 softmax, probabilities cast to bf16 and transposed on the PE four
    blocks per PSUM copy, AV matmul over PAIRED query blocks (moving dim 256
    so bf16 runs 1 cyc/row); y reshards token-wise via ONE bf16 AllToAll.
  Stage C (token-sharded): full wo (bf16, prefetched during attention) on
    this core's 256 tokens + residual from the stage-A x shard; the ffn
    rmsnorm reduction is core-LOCAL (no AllReduce); res1 ships straight out
    as a kernel output and the final residual add happens on the host during
    unsharding (0.002% of the FLOPs).
  MLP (data-parallel): full SwiGLU on this core's 256 tokens in bf16,
    streaming the full w1/w3 j-tiles and w2 k-slabs (no h2 AllGather, no
    ReduceScatter); phase 2 uses g as the stationary operand so the output
    lands token-major without transposes.

Only 3 collectives total (agq, agkv, y AllToAll).  All rmsnorm weights are
folded into the following weight matrix on the host (mathematically exact);
softmax scale (-96) is folded into q at the wq_b eviction; rmsnorm
reduce+broadcast is one all-ones 128x128 fp32r matmul.  Softmax statistics
and all PSUM accumulation stay fp32.  fp16 (not bf16) is used for the
latent/QK path: the softscale (-96) amplifies latent rounding into argmax
flips, and bf16's 8-bit mantissa measurably breaches the error budget while
fp16's 10-bit mantissa keeps the final rel-err ~8e-3 (gate 2e-2).
"""

import os
import sys

sys.path.insert(0, "/opt/trn_rl_repo")

from contextlib import ExitStack

import numpy as np
import ml_dtypes

import concourse.bacc as bacc
import concourse.bass as bass
import concourse.mybir as mybir
import concourse.tile as tile
from concourse.bass_utils import run_bass_kernel_spmd
from concourse.masks import make_identity

F32 = mybir.dt.float32
F32R = mybir.dt.float32r
F16 = mybir.dt.float16
BF16 = mybir.dt.bfloat16
AX = mybir.AxisListType.X
ADD = mybir.AluOpType.add
SUB = mybir.AluOpType.subtract
MUL = mybir.AluOpType.mult
AF = mybir.ActivationFunctionType

NCORES = 8
B, T, D = 2, 1024, 2048
H = 16
NOPE, ROPE = 128, 64
QKD = NOPE + ROPE  # 192
QLORA, KVLORA = 1536, 512
VHD = 128
INTER = 8192
EPS = 1e-6
SOFTSCALE = float(QKD) * -0.5  # -96.0

N_TOK = B * T  # 2048
S = N_TOK // NCORES  # 256 tokens per core (stage A shard)
HPC = H // NCORES  # 2 heads per core
DO = D // NCORES  # 256 output-feature rows per core
ISH = INTER // NCORES  # 1024 intermediate rows per core
AGQ = QLORA
AGKV = KVLORA + ROPE  # 576

# exact-causal attention geometry: query row-block sc (128 rows) attends to
# key columns [0, 128*(sc+1)); we round the attended width up to 256 so every
# matmul's moving dim is >=256 (1 cyc/row in fp32r/bf16).  Paired row-blocks
# (2*p, 2*p+1) share the same width so their probability transposes can be
# packed side by side and the AV matmul runs with a 256-wide moving dim.
WPAIR = [256, 512, 768, 1024]
# per-sc (col_offset, width) of the single chunk that contains above-diagonal
# elements and therefore needs the additive mask; all other computed chunks
# are strictly below the diagonal and all skipped chunks are fully masked.
MASKCHUNK = [
    (0, 256), (0, 256), (0, 512), (0, 512),
    (512, 256), (512, 256), (512, 512), (512, 512),
]


def _chunks_of(w):
    """Split width w into 512-col chunks with a possible 256 tail."""
    out = []
    c = 0
    while c < w:
        step = 512 if w - c >= 512 else (w - c)
        out.append((c, step))
        c += step
    return out


_CACHED_NC = None
LAST_RESULTS = None  # test.py reads these
LAST_IN_MAPS = None


def _bc(ap):
    """View an fp32 DRAM AP as fp32r for a weight-slab DMA."""
    return ap.bitcast(F32R)


def _rms_inv(nc, ones128, eps_ap, psum_pool, work_pool, chunks, dim, n, tag):
    """chunks: list of APs [128, n] covering `dim` feature rows (feature-major).
    Returns an SBUF tile [128, n] whose every row is 1/sqrt(mean_sq + eps).
    ones128 must be an fp32r tile; the squares are written fp32r so the
    reducing matmul runs 1 cyc/row."""
    ss = psum_pool.tile([128, n], F32, tag="rms_ss", name=f"{tag}_ss")
    nchunks = len(chunks)
    for i, xc in enumerate(chunks):
        xx = work_pool.tile([128, n], F32R, tag="rms_xx", name=f"{tag}_xx")
        nc.scalar.square(xx[:], xc)
        nc.tensor.matmul(
            ss[:], ones128[:], xx[:], start=(i == 0), stop=(i == nchunks - 1)
        )
    std = work_pool.tile([128, n], F32, tag="rms_std", name=f"{tag}_std")
    nc.scalar.activation(std[:], ss[:], AF.Sqrt, bias=eps_ap, scale=1.0 / dim)
    inv = work_pool.tile([128, n], F32, tag=f"{tag}_inv", name=f"{tag}_inv")
    nc.vector.reciprocal(inv[:], std[:])
    return inv


def _rope(nc, pool, out64, in64, cos, sin, n, tag, dt=F32):
    """out64/in64: APs [64, n]; rows 0:32 = even lanes, 32:64 = odd lanes.
    cos/sin: APs [32, n] at partition base 0.  in64 may sit at any 32-aligned
    base; walrus requires equal input bases for 2-input SBUF ops, so stage the
    halves through base-0 copies first (single-input ops may shift bases)."""
    qe = pool.tile([32, n], dt, tag="rope_qe", name=f"{tag}_qe")
    qo = pool.tile([32, n], dt, tag="rope_qo", name=f"{tag}_qo")
    nc.scalar.copy(qe[:], in64[0:32, :])
    nc.scalar.copy(qo[:], in64[32:64, :])
    t0 = pool.tile([32, n], dt, tag="rope_t0", name=f"{tag}_t0")
    t1 = pool.tile([32, n], dt, tag="rope_t1", name=f"{tag}_t1")
    nc.vector.tensor_tensor(t0[:], qe[:], cos, MUL)
    nc.vector.tensor_tensor(t1[:], qo[:], sin, MUL)
    nc.vector.tensor_tensor(out64[0:32, :], t0[:], t1[:], SUB)
    t2 = pool.tile([32, n], dt, tag="rope_t2", name=f"{tag}_t2")
    t3 = pool.tile([32, n], dt, tag="rope_t3", name=f"{tag}_t3")
    nc.vector.tensor_tensor(t2[:], qe[:], sin, MUL)
    nc.vector.tensor_tensor(t3[:], qo[:], cos, MUL)
    nc.vector.tensor_tensor(out64[32:64, :], t2[:], t3[:], ADD)


def _build():
    nc = bacc.Bacc("TRN2", target_bir_lowering=False, debug=False, num_devices=NCORES)
    RG = [list(range(NCORES))]

    # ---- kernel parameters (per-core data supplied via in_maps) ----
    p_xs = nc.declare_dram_parameter("xs", [D, S], F32, isOutput=False)
    p_mask = nc.declare_dram_parameter("mask", [T, T], F32, isOutput=False)
    p_cosq = nc.declare_dram_parameter("cosq", [32, T], F16, isOutput=False)
    p_sinq = nc.declare_dram_parameter("sinq", [32, T], F16, isOutput=False)
    p_cosk = nc.declare_dram_parameter("cosk", [32, S], F32, isOutput=False)
    p_sink = nc.declare_dram_parameter("sink", [32, S], F32, isOutput=False)
    p_wqa = nc.declare_dram_parameter("wqaT", [D, QLORA], F16, isOutput=False)
    p_wqb = nc.declare_dram_parameter("wqbT", [QLORA, HPC * QKD], F16, isOutput=False)
    p_wkva = nc.declare_dram_parameter("wkvaT", [D, AGKV], F16, isOutput=False)
    p_wkb = nc.declare_dram_parameter("wkbT", [KVLORA, HPC * NOPE], F16, isOutput=False)
    p_wv = nc.declare_dram_parameter("wvT", [KVLORA, HPC * VHD], F16, isOutput=False)
    p_wo = nc.declare_dram_parameter("woT", [H * VHD, D], BF16, isOutput=False)
    p_w13 = nc.declare_dram_parameter("w13T", [D, 2 * INTER], BF16, isOutput=False)
    p_w2 = nc.declare_dram_parameter("w2T", [INTER, D], BF16, isOutput=False)
    p_mlp = nc.declare_dram_parameter("mlpT", [S, D], BF16, isOutput=True)
    p_res1 = nc.declare_dram_parameter("res1", [D, S], F32, isOutput=True)

    with tile.TileContext(
        nc, trace_sim=bool(os.environ.get("TILE_TRACE_SIM"))
    ) as tc, ExitStack() as root:
        dram = root.enter_context(tc.tile_pool(name="dram", bufs=1, space="DRAM"))
        agq_in = dram.tile([AGQ, S], F16, name="agq_in")
        agq_out = dram.tile([NCORES * AGQ, S], F16, addr_space="Shared", name="agq_out")
        agkv_in = dram.tile([AGKV, S], F16, name="agkv_in")
        agkv_out = dram.tile(
            [NCORES * AGKV, S], F16, addr_space="Shared", name="agkv_out"
        )
        y2a_in = dram.tile([H * VHD, S], BF16, name="y2a_in")
        y2a_out = dram.tile([H * VHD, S], BF16, name="y2a_out")

        const = root.enter_context(tc.tile_pool(name="const", bufs=1))
        onesf = const.tile([128, 128], F32, name="onesf")
        nc.vector.memset(onesf[:], 1.0)
        ones128 = const.tile([128, 128], F32R, name="ones128")
        nc.vector.tensor_copy(ones128[:], onesf[:])
        eps_sb = const.tile([128, 1], F32, name="eps_sb")
        nc.vector.memset(eps_sb[:], EPS)
        ident = const.tile([128, 128], BF16, name="ident")
        identf = const.tile([128, 128], F32, name="identf")
        make_identity(nc, identf[:])
        nc.vector.tensor_copy(ident[:], identf[:])
        cosq = const.tile([32, T], F16, name="cosq")
        sinq = const.tile([32, T], F16, name="sinq")
        nc.sync.dma_start(cosq[:], p_cosq[:, :])
        nc.sync.dma_start(sinq[:], p_sinq[:, :])

        # =========================== Stage A ===========================
        xroot = root.enter_context(tc.tile_pool(name="xroot", bufs=1))
        x_sb = xroot.tile([128, D // 128, S], F32, name="x_sb")

        wo_sb = xroot.tile([128, H * VHD // 128, D], BF16, name="wo_sb")
        with ExitStack() as sa:
            a_res = sa.enter_context(tc.tile_pool(name="a_res", bufs=1))
            a_wk = sa.enter_context(tc.tile_pool(name="a_wk", bufs=3))
            a_w = sa.enter_context(tc.tile_pool(name="a_w", bufs=3))
            a_ps2 = sa.enter_context(tc.tile_pool(name="a_ps2", bufs=2, space="PSUM"))

            for xq in range(4):
                nc.sync.dma_start(
                    x_sb[:, 4 * xq : 4 * (xq + 1), :],
                    p_xs[512 * xq : 512 * (xq + 1), :].rearrange(
                        "(ko p) n -> p ko n", p=128
                    ),
                )
            # The attn rmsnorm is a per-token scale; both the q and kv latent
            # paths re-normalize (rmsnorm) right after their linear projection,
            # so the scale cancels exactly (up to the negligible eps shift) and
            # the projections can run on RAW x.  Only the 64 decoupled-rope
            # k_pe rows keep the scale, applied after their matmul via inv_a
            # (whose rows are all equal thanks to the ones-matmul reduce).
            xh = a_res.tile([128, D // 128, S], F16, name="xh")
            for k in range(D // 128):
                nc.vector.tensor_copy(xh[:, k, :], x_sb[:, k, :])
            inv_a = _rms_inv(
                nc, ones128, eps_sb[:], a_ps2, a_wk,
                [x_sb[:, k, :] for k in range(D // 128)], D, S, "rmsa",
            )

            # q_lat = wq_a' @ xh -> [1536, S]; two m-groups of 6 accumulators
            # (group 0 is copied to SBUF so its banks free up for group 1)
            qn = QLORA // 128  # 12
            ql_parts = []
            with ExitStack() as sq:
                q_ps = sq.enter_context(
                    tc.tile_pool(name="q_ps", bufs=1, space="PSUM")
                )
                for g in range(2):
                    qps = [
                        q_ps.tile([128, S], F32, tag=f"qm{mi}", name=f"ql_ps{g}{mi}")
                        for mi in range(6)
                    ]
                    for k in range(D // 128):
                        wt = a_w.tile([128, 768], F16, tag="wqa_slab", name="wqa_t")
                        nc.sync.dma_start(
                            wt[:],
                            p_wqa[128 * k : 128 * (k + 1), 768 * g : 768 * (g + 1)],
                        )
                        for mi in range(6):
                            nc.tensor.matmul(
                                qps[mi][:], wt[:, 128 * mi : 128 * (mi + 1)],
                                xh[:, k, :],
                                start=(k == 0), stop=(k == D // 128 - 1),
                            )
                    if g == 0:
                        for mi in range(6):
                            qs = a_wk.tile(
                                [128, S], F32, tag=f"qsb{mi}", name=f"qsb{mi}"
                            )
                            nc.any.tensor_copy(qs[:], qps[mi][:])
                            ql_parts.append(qs)
                    else:
                        ql_parts.extend(qps)
                inv_q = _rms_inv(
                    nc, ones128, eps_sb[:], a_ps2, a_wk,
                    [ql_parts[m][:] for m in range(qn)], QLORA, S, "rmsq",
                )
                for m in range(qn):
                    qh = a_wk.tile([128, S], F16, tag="qh", name="qh")
                    nc.vector.tensor_tensor(qh[:], ql_parts[m][:], inv_q[:], MUL)
                    nc.sync.dma_start(agq_in[128 * m : 128 * (m + 1), :], qh[:])
            nc.gpsimd.collective_compute(
                "AllGather", mybir.AluOpType.bypass, replica_groups=RG,
                ins=[agq_in[:].opt()], outs=[agq_out[:].opt()],
            )
            # kv_all = wkv_a' @ xh -> [576, S]; rows 0:512 latent, 512:576 rope
            # k-slab weight DMAs (full-bw lines) + 5 live psum accumulators
            mchunks = [(0, 128), (128, 128), (256, 128), (384, 128), (512, 64)]
            with ExitStack() as skv:
                kv_ps = skv.enter_context(
                    tc.tile_pool(name="kv_ps", bufs=1, space="PSUM")
                )
                kvps = [
                    kv_ps.tile([128, S], F32, tag=f"kvm{mi}", name=f"kv_ps{mi}")
                    for mi in range(len(mchunks))
                ]
                for k in range(D // 128):
                    wt = a_w.tile([128, AGKV], F16, tag="wkva_slab", name="wkva_t")
                    nc.sync.dma_start(wt[:], p_wkva[128 * k : 128 * (k + 1), :])
                    for mi, (moff, msz) in enumerate(mchunks):
                        nc.tensor.matmul(
                            kvps[mi][:msz, :], wt[:, moff : moff + msz],
                            xh[:, k, :],
                            start=(k == 0), stop=(k == D // 128 - 1),
                        )
                inv_kv = _rms_inv(
                    nc, ones128, eps_sb[:], a_ps2, a_wk,
                    [kvps[m][:] for m in range(4)], KVLORA, S, "rmskv",
                )
                for m in range(4):
                    ln = a_wk.tile([128, S], F16, tag="latn", name="latn")
                    nc.vector.tensor_tensor(ln[:], kvps[m][:], inv_kv[:], MUL)
                    nc.sync.dma_start(agkv_in[128 * m : 128 * (m + 1), :], ln[:])
                # rope the decoupled key (shared across heads), feature-permuted
                # on host; computed fp32 from psum, stored fp16
                cosk = a_res.tile([32, S], F32, name="cosk")
                sink = a_res.tile([32, S], F32, name="sink")
                nc.sync.dma_start(cosk[:], p_cosk[:, :])
                nc.sync.dma_start(sink[:], p_sink[:, :])
                kpe_s = a_wk.tile([64, S], F32, tag="kpe_s", name="kpe_s")
                nc.vector.tensor_tensor(
                    kpe_s[:], kvps[4][0:64, :], inv_a[0:64, :], MUL
                )
                kpe_r = a_wk.tile([64, S], F32, tag="kpe_r", name="kpe_r")
                _rope(
                    nc, a_wk, kpe_r[:], kpe_s[:], cosk[:], sink[:], S, "ropek"
                )
                kpe_h = a_wk.tile([64, S], F16, tag="kpe_h", name="kpe_h")
                nc.scalar.copy(kpe_h[:], kpe_r[:])
                nc.sync.dma_start(
                    agkv_in[KVLORA : KVLORA + ROPE, :], kpe_h[:]
                )
            nc.gpsimd.collective_compute(
                "AllGather", mybir.AluOpType.bypass, replica_groups=RG,
                ins=[agkv_in[:].opt()], outs=[agkv_out[:].opt()],
            )

        # =========================== Stage B ===========================
        with ExitStack() as sb:
            b_res = sb.enter_context(tc.tile_pool(name="b_res", bufs=1))
            b_wk = sb.enter_context(tc.tile_pool(name="b_wk", bufs=2))

            k_sb = b_res.tile([128, HPC, N_TOK], F16, name="k_sb")
            kpe_sb = b_res.tile([128, N_TOK], F16, name="kpe_sb")
            v_sb = b_res.tile([128, N_TOK // 128, HPC * VHD], BF16, name="v_sb")
            q_sb = b_res.tile([128, 3, N_TOK], F16, name="q_sb")
            qpe_sb = b_res.tile([128, N_TOK], F16, name="qpe_sb")

            # ---- q for the 2 local heads (fold SOFTSCALE here) ----
            with ExitStack() as s2:
                b_w2 = s2.enter_context(tc.tile_pool(name="b_w2", bufs=1))
                b_rhs2 = s2.enter_context(tc.tile_pool(name="b_rhs2", bufs=2))
                b2_ps = s2.enter_context(
                    tc.tile_pool(name="b2_ps", bufs=2, space="PSUM")
                )
                wqb_sb = b_w2.tile([128, QLORA // 128, HPC * QKD], F16, name="wqb_sb")
                nc.sync.dma_start(
                    wqb_sb[:], p_wqb.rearrange("(ko p) m -> p ko m", p=128)
                )
                for blk in range(NCORES):
                    base = AGQ * blk
                    qrhs = b_rhs2.tile([128, QLORA // 128, S], F16, tag="qrhs", name="qrhs")
                    nc.sync.dma_start(
                        qrhs[:],
                        agq_out[base : base + QLORA, :].rearrange(
                            "(ko p) n -> p ko n", p=128
                        ),
                    )
                    psq = [
                        b2_ps.tile([128, S], F32, tag=f"psq{m}", name=f"psq{m}")
                        for m in range(3)
                    ]
                    for k in range(QLORA // 128):
                        for m in range(3):
                            nc.tensor.matmul(
                                psq[m][:], wqb_sb[:, k, 128 * m : 128 * (m + 1)],
                                qrhs[:, k, :],
                                start=(k == 0), stop=(k == QLORA // 128 - 1),
                            )
                    for m in range(3):
                        nc.scalar.mul(
                            q_sb[:, m, S * blk : S * (blk + 1)], psq[m][:], SOFTSCALE
                        )

            # ---- expand k_nope and v (token-major) for the 2 local heads ----
            with ExitStack() as s1:
                b_w1 = s1.enter_context(tc.tile_pool(name="b_w1", bufs=1))
                b_rhs1 = s1.enter_context(tc.tile_pool(name="b_rhs1", bufs=2))
                b1_ps = s1.enter_context(
                    tc.tile_pool(name="b1_ps", bufs=2, space="PSUM")
                )
                wkb_sb = b_w1.tile([128, 4, HPC * NOPE], F16, name="wkb_sb")
                nc.sync.dma_start(
                    wkb_sb[:], p_wkb.rearrange("(ko p) m -> p ko m", p=128)
                )
                wv_sb = b_w1.tile([128, 4, HPC * VHD], F16, name="wv_sb")
                nc.sync.dma_start(
                    wv_sb[:], p_wv.rearrange("(ko p) m -> p ko m", p=128)
                )
                for blk in range(NCORES):
                    base = AGKV * blk
                    lat = b_rhs1.tile([128, 4, S], F16, tag="lat", name="lat")
                    nc.sync.dma_start(
                        lat[:],
                        agkv_out[base : base + KVLORA, :].rearrange(
                            "(ko p) n -> p ko n", p=128
                        ),
                    )
                    for m in range(HPC):
                        psk = b1_ps.tile([128, S], F32, tag="psk", name="psk")
                        for k in range(4):
                            nc.tensor.matmul(
                                psk[:], wkb_sb[:, k, 128 * m : 128 * (m + 1)],
                                lat[:, k, :],
                                start=(k == 0), stop=(k == 3),
                            )
                        nc.any.tensor_copy(k_sb[:, m, S * blk : S * (blk + 1)], psk[:])
                    for th in range(S // 128):
                        psv = b1_ps.tile([128, HPC * VHD], F32, tag="psv", name="psv")
                        for k in range(4):
                            nc.tensor.matmul(
                                psv[:], lat[:, k, 128 * th : 128 * (th + 1)],
                                wv_sb[:, k, :], start=(k == 0), stop=(k == 3),
                            )
                        nc.any.tensor_copy(v_sb[:, (S // 128) * blk + th, :], psv[:])
                    nc.sync.dma_start(
                        kpe_sb[0:64, S * blk : S * (blk + 1)],
                        agkv_out[base + KVLORA : base + KVLORA + ROPE, :],
                    )
                    nc.sync.dma_start(
                        kpe_sb[64:128, S * blk : S * (blk + 1)],
                        agkv_out[base + KVLORA : base + KVLORA + ROPE, :],
                    )

            # rope q_pe: q_sb chunk 2 = [h0_e, h0_o, h1_e, h1_o] x32 rows
            with tc.tile_pool(name="rope_wk", bufs=1) as rp:
                for b in range(B):
                    for h in range(HPC):
                        _rope(
                            nc, rp,
                            qpe_sb[64 * h : 64 * (h + 1), T * b : T * (b + 1)],
                            q_sb[64 * h : 64 * (h + 1), 2, T * b : T * (b + 1)],
                            cosq[:, :], sinq[:, :], T, f"ropeq{b}{h}", dt=F16,
                        )

            # prefetch the full wo (consumed at stage C) during attention, in
            # 16 chunks so stage A/B loads can interleave instead of
            # head-blocking behind one 25us transfer
            for k in range(H * VHD // 128):
                nc.sync.dma_start(wo_sb[:, k, :], p_wo[128 * k : 128 * (k + 1), :])

            # ---- mask: one diagonal chunk per query row-block, shared h/b ----
            mtile = b_res.tile([128, T // 128, 512], F32, name="mtile")
            for sc in range(T // 128):
                c0, w = MASKCHUNK[sc]
                nc.sync.dma_start(
                    mtile[:, sc, 0:w],
                    p_mask[128 * sc : 128 * (sc + 1), c0 : c0 + w],
                )

            # ---- attention: exact-causal, paired row-blocks for 256-wide AV ----
            with ExitStack() as s3:
                b3_ps = s3.enter_context(
                    tc.tile_pool(name="b3_ps", bufs=2, space="PSUM")
                )
                for h in range(HPC):
                    for pair in range(T // 256):
                        W = WPAIR[pair]
                        ntj = W // 128
                        for b in range(B):
                            ptrp = b_wk.tile(
                                [128, T // 128, 256], BF16, tag="ptrp", name="ptrp"
                            )
                            for blk2 in range(2):
                                sc = 2 * pair + blk2
                                s0 = T * b + 128 * sc
                                scs = b_wk.tile([128, T], F32, tag="scs", name="scs")
                                for (c0, w) in _chunks_of(W):
                                    t0 = T * b + c0
                                    ps = b3_ps.tile(
                                        [128, 512], F32, tag="ps_qk", name="ps_qk"
                                    )
                                    nc.tensor.matmul(
                                        ps[:, :w],
                                        q_sb[:, h, s0 : s0 + 128],
                                        k_sb[:, h, t0 : t0 + w],
                                        start=True, stop=False,
                                    )
                                    nc.tensor.matmul(
                                        ps[:, :w],
                                        qpe_sb[64 * h : 64 * (h + 1), s0 : s0 + 128],
                                        kpe_sb[64 * h : 64 * (h + 1), t0 : t0 + w],
                                        start=False, stop=True,
                                    )
                                    if (c0, w) == MASKCHUNK[sc]:
                                        nc.vector.tensor_tensor(
                                            scs[:, c0 : c0 + w], ps[:, :w],
                                            mtile[:, sc, 0:w], ADD,
                                        )
                                    else:
                                        nc.any.tensor_copy(
                                            scs[:, c0 : c0 + w], ps[:, :w]
                                        )
                                nmax = b_wk.tile([128, 1], F32, tag="nmax", name="nmax")
                                nc.vector.reduce_max(
                                    nmax[:], scs[:, :W], axis=AX, negate=True
                                )
                                p_sb = b_wk.tile([128, T], F32, tag="p_sb", name="p_sb")
                                zsum = b_wk.tile([128, 1], F32, tag="zsum", name="zsum")
                                nc.scalar.activation(
                                    p_sb[:, :W], scs[:, :W], AF.Exp,
                                    bias=nmax[:], accum_out=zsum[:],
                                )
                                invz = b_wk.tile([128, 1], F32, tag="invz", name="invz")
                                nc.vector.reciprocal(invz[:], zsum[:])
                                pn = b_wk.tile([128, T], BF16, tag="pn", name="pn")
                                nc.vector.tensor_scalar_mul(
                                    pn[:, :W], p_sb[:, :W], invz[:]
                                )
                                for tj0 in range(0, ntj, 4):
                                    tjn = min(4, ntj - tj0)
                                    pst = b3_ps.tile(
                                        [128, 4, 128], BF16, tag="pst", name="pst"
                                    )
                                    for i in range(tjn):
                                        nc.tensor.transpose(
                                            pst[:, i, :],
                                            pn[:, 128 * (tj0 + i) : 128 * (tj0 + i + 1)],
                                            ident[:],
                                        )
                                    nc.any.tensor_copy(
                                        ptrp[
                                            :, tj0 : tj0 + tjn,
                                            128 * blk2 : 128 * (blk2 + 1),
                                        ],
                                        pst[:, 0:tjn, :],
                                    )
                            psy = b3_ps.tile([128, 256], F32, tag="psy", name="psy")
                            for tj in range(ntj):
                                nc.tensor.matmul(
                                    psy[:],
                                    v_sb[:, (T // 128) * b + tj, VHD * h : VHD * (h + 1)],
                                    ptrp[:, tj, :],
                                    start=(tj == 0), stop=(tj == ntj - 1),
                                )
                            yst = b_wk.tile([128, 256], BF16, tag="yst", name="yst")
                            nc.any.tensor_copy(yst[:], psy[:])
                            d = 4 * b + pair
                            nc.sync.dma_start(
                                y2a_in[256 * d + 128 * h : 256 * d + 128 * (h + 1), :],
                                yst[:],
                            )

            # AllToAll chunk d (rows [256d, 256d+256)) went to core d: my 2
            # heads x d's 256 tokens (written per-block during attention).
            # Received chunk e = e's heads 2e,2e+1 for MY tokens -> y2a_out is
            # exactly [all 16 heads' vhd, my tokens].
            nc.gpsimd.collective_compute(
                "AllToAll", mybir.AluOpType.bypass, replica_groups=RG,
                ins=[y2a_in[:].opt()], outs=[y2a_out[:].opt()],
            )

        # ====== Stage C: token-sharded wo + LOCAL ffn-norm (no AllReduce) ======
        # Each core computes attn-out/res1/h2 for its OWN 256 tokens over the
        # full feature dim: the ffn rmsnorm reduction is core-local, h2 feeds a
        # DATA-PARALLEL MLP directly (no h2 AllGather, no ReduceScatter), and
        # res1 ships straight out as a kernel output (host does the final add).
        h2root = root.enter_context(tc.tile_pool(name="h2root", bufs=1))
        h2n = h2root.tile([128, D // 128, S], BF16, name="h2n")
        with ExitStack() as sc_stack:
            c_res = sc_stack.enter_context(tc.tile_pool(name="c_res", bufs=1))
            c_wk = sc_stack.enter_context(tc.tile_pool(name="c_wk", bufs=3))
            c_ps = sc_stack.enter_context(
                tc.tile_pool(name="c_ps", bufs=2, space="PSUM")
            )

            yloc = c_res.tile([128, H * VHD // 128, S], BF16, name="yloc")
            nc.sync.dma_start(
                yloc[:], y2a_out[:].rearrange("(ko p) n -> p ko n", p=128)
            )
            res1f = c_res.tile([128, D // 128, S], F32, name="res1f")
            for dm in range(D // 128):
                psD = c_ps.tile([128, S], F32, tag="psD", name="psD")
                for k in range(H * VHD // 128):
                    nc.tensor.matmul(
                        psD[:], wo_sb[:, k, 128 * dm : 128 * (dm + 1)], yloc[:, k, :],
                        start=(k == 0), stop=(k == H * VHD // 128 - 1),
                    )
                nc.vector.tensor_tensor(res1f[:, dm, :], psD[:], x_sb[:, dm, :], ADD)

            inv_f = _rms_inv(
                nc, ones128, eps_sb[:], c_ps, c_wk,
                [res1f[:, k, :] for k in range(D // 128)], D, S, "rmsf",
            )
            for k in range(D // 128):
                nc.vector.tensor_tensor(h2n[:, k, :], res1f[:, k, :], inv_f[:], MUL)
            # res1 goes straight out as a kernel output; the final residual
            # add happens on the host during unsharding (0.002% of the FLOPs).
            nc.sync.dma_start(p_res1.rearrange("(ko p) n -> p ko n", p=128), res1f[:])

        # ---- DATA-PARALLEL MLP: full SwiGLU on this core's 256 tokens ----
        # Streams the full w1/w3 (j-tiles) and w2 (k-slabs) in bf16; all
        # matmuls have a 256-wide moving dim (the token count).  Phase 2
        # computes the output TOKEN-major (tokens on partitions) by using g as
        # the stationary operand, so the mlpT output needs no transposes.
        NJ = 2 * INTER // 256  # 64 w1/w3 j-tiles
        NK = INTER // 128  # 64 w2 k-slabs
        with ExitStack() as sm:
            m_res = sm.enter_context(tc.tile_pool(name="m_res", bufs=1))
            m_w = sm.enter_context(tc.tile_pool(name="m_w", bufs=3))
            m_wk = sm.enter_context(tc.tile_pool(name="m_wk", bufs=3))
            g_sb = m_res.tile([128, NJ, S], BF16, name="g_sb")
            with ExitStack() as sm1:
                m_ps1 = sm1.enter_context(
                    tc.tile_pool(name="m_ps1", bufs=2, space="PSUM")
                )
                for j in range(NJ):
                    wj = m_w.tile([128, D // 128, 256], BF16, tag="wj", name="wj")
                    nc.sync.dma_start(
                        wj[:],
                        p_w13[:, 256 * j : 256 * (j + 1)].rearrange(
                            "(ko p) m -> p ko m", p=128
                        ),
                    )
                    psa = m_ps1.tile([128, S], F32, tag="psa", name="psa")
                    psb = m_ps1.tile([128, S], F32, tag="psb", name="psb")
                    for k in range(D // 128):
                        nc.tensor.matmul(
                            psa[:], wj[:, k, 0:128], h2n[:, k, :],
                            start=(k == 0), stop=(k == D // 128 - 1),
                        )
                        nc.tensor.matmul(
                            psb[:], wj[:, k, 128:256], h2n[:, k, :],
                            start=(k == 0), stop=(k == D // 128 - 1),
                        )
                    tsi = m_wk.tile([128, S], F32, tag="tsi", name="tsi")
                    nc.scalar.activation(tsi[:], psa[:], AF.Silu)
                    nc.vector.tensor_tensor(g_sb[:, j, :], tsi[:], psb[:], MUL)
            with ExitStack() as sm2:
                m_ps2 = sm2.enter_context(
                    tc.tile_pool(name="m_ps2", bufs=1, space="PSUM")
                )
                pso = [
                    m_ps2.tile([128, 512], F32, tag=f"pso{tb}_{dc}", name=f"pso{tb}{dc}")
                    for tb in range(S // 128)
                    for dc in range(D // 512)
                ]
                for k in range(NK):
                    w2k = m_w.tile([128, D], BF16, tag="w2k", name="w2k")
                    nc.sync.dma_start(w2k[:], p_w2[128 * k : 128 * (k + 1), :])
                    for tb in range(S // 128):
                        for dc in range(D // 512):
                            nc.tensor.matmul(
                                pso[tb * (D // 512) + dc][:],
                                g_sb[:, k, 128 * tb : 128 * (tb + 1)],
                                w2k[:, 512 * dc : 512 * (dc + 1)],
                                start=(k == 0), stop=(k == NK - 1),
                            )
                for tb in range(S // 128):
                    for dc in range(D // 512):
                        o_sb = m_wk.tile([128, 512], BF16, tag="o_sb", name="o_sb")
                        nc.any.tensor_copy(o_sb[:], pso[tb * (D // 512) + dc][:])
                        nc.sync.dma_start(
                            p_mlp[
                                128 * tb : 128 * (tb + 1), 512 * dc : 512 * (dc + 1)
                            ],
                            o_sb[:],
                        )

    nc.compile()
    return nc


def _rope_perm(n):
    """Permutation putting even lanes first then odd lanes, for an n-row rope
    block (n even): [0,2,4,...,n-2, 1,3,5,...,n-1]."""
    return np.concatenate([np.arange(0, n, 2), np.arange(1, n, 2)])


def kernel(**inputs):
    global _CACHED_NC, LAST_RESULTS, LAST_IN_MAPS
    f32 = lambda a: np.ascontiguousarray(np.asarray(a), dtype=np.float32)
    f16 = lambda a: np.ascontiguousarray(np.asarray(a), dtype=np.float16)
    bf16 = lambda a: np.ascontiguousarray(np.asarray(a), dtype=ml_dtypes.bfloat16)

    x = f32(inputs["x"]).reshape(N_TOK, D)
    mask = f32(inputs["mask"])
    cos = f32(inputs["freqs_cos"])  # [T, 32]
    sin = f32(inputs["freqs_sin"])
    attn_nw = f32(inputs["attn_norm_w"])
    wq_a = f32(inputs["wq_a"]) * attn_nw[None, :]
    q_nw = f32(inputs["q_norm_w"])
    wq_b = f32(inputs["wq_b"]) * q_nw[None, :]
    wkv_a = f32(inputs["wkv_a"]) * attn_nw[None, :]
    kv_nw = f32(inputs["kv_norm_w"])
    wkv_b = f32(inputs["wkv_b"]) * kv_nw[None, :]
    wo = f32(inputs["wo"])
    ffn_nw = f32(inputs["ffn_norm_w"])
    w1 = f32(inputs["w1"]) * ffn_nw[None, :]
    w3 = f32(inputs["w3"]) * ffn_nw[None, :]
    w2 = f32(inputs["w2"])

    xT = np.ascontiguousarray(x.T)  # [D, N_TOK] feature-major
    cosT = np.ascontiguousarray(cos.T)  # [32, T]
    sinT = np.ascontiguousarray(sin.T)

    # wkv_a rows: keep 0:512 (latent); permute rope rows 512:576 to even|odd
    pk = _rope_perm(ROPE)
    wkva_p = wkv_a.copy()
    wkva_p[KVLORA:] = wkv_a[KVLORA:][pk]
    wkvaT = np.ascontiguousarray(wkva_p.T)  # [D, 576]

    wqaT = np.ascontiguousarray(wq_a.T)  # [D, 1536]
    # full wo, transposed, natural hv row order (AllToAll lands head k's y at
    # row block k); shared across cores
    woT = bf16(wo.T)  # [2048, 2048]
    # full w1/w3 interleaved in 256-col j-tiles (128 w1 | 128 w3), and full
    # w2 transposed; shared across cores (data-parallel MLP)
    w13T = np.empty((D, 2 * INTER), np.float32)
    for j in range(INTER // 128):
        w13T[:, 256 * j : 256 * j + 128] = w1[128 * j : 128 * (j + 1)].T
        w13T[:, 256 * j + 128 : 256 * (j + 1)] = w3[128 * j : 128 * (j + 1)].T
    w13T = bf16(w13T)
    w2T = bf16(w2.T)  # [8192, 2048]

    in_maps = []
    for c in range(NCORES):
        heads = [HPC * c + j for j in range(HPC)]
        # wq_b rows per head h: h*QKD .. h*QKD+192 (128 nope + 64 rope)
        # target col order: [h0_nope(128), h1_nope(128), h0_rope_eo(64), h1_rope_eo(64)]
        cols = []
        for h in heads:
            cols.append(wq_b[h * QKD : h * QKD + NOPE])
        for h in heads:
            cols.append(wq_b[h * QKD + NOPE : (h + 1) * QKD][pk])
        wqbT = f16(np.concatenate(cols, axis=0).T)  # [1536, 384]

        # wkv_b rows per head h: h*(NOPE+VHD) + [0:128]=k_nope, [128:256]=v
        kw = np.concatenate(
            [wkv_b[h * (NOPE + VHD) : h * (NOPE + VHD) + NOPE] for h in heads], axis=0
        )
        vw = np.concatenate(
            [wkv_b[h * (NOPE + VHD) + NOPE : (h + 1) * (NOPE + VHD)] for h in heads],
            axis=0,
        )
        wkbT = f16(kw.T)  # [512, 256]
        wvT = f16(vw.T)  # [512, 256]


        tpos = (S * c) % T  # position within batch of this token shard
        in_maps.append(
            {
                "xs": np.ascontiguousarray(xT[:, S * c : S * (c + 1)]),
                "mask": mask,
                "cosq": f16(cosT),
                "sinq": f16(sinT),
                "cosk": np.ascontiguousarray(cosT[:, tpos : tpos + S]),
                "sink": np.ascontiguousarray(sinT[:, tpos : tpos + S]),
                "wqaT": f16(wqaT),
                "wqbT": wqbT,
                "wkvaT": f16(wkvaT),
                "wkbT": wkbT,
                "wvT": wvT,
                "woT": woT,
                "w13T": w13T,
                "w2T": w2T,
            }
        )

    LAST_IN_MAPS = in_maps
    if _CACHED_NC is None:
        _CACHED_NC = _build()
    nc = _CACHED_NC

    trace = bool(os.environ.get("KERNEL_TRACE"))
    res = run_bass_kernel_spmd(
        nc, in_maps, core_ids=list(range(NCORES)), trace=trace
    )
    LAST_RESULTS = res

    # final residual add during unsharding: both mlpT [S, D] (token-major)
    # and res1 [D, S] (feature-major) are token-sharded per core
    out = np.concatenate(
        [
            np.asarray(res.results[c]["mlpT"]).astype(np.float32)
            + np.asarray(res.results[c]["res1"]).T
            for c in range(NCORES)
        ],
        axis=0,
    )
    return np.ascontiguousarray(out).reshape(B, T, D).astype(np.float32)


# revision 31
# speedup vs baseline: 2.3696x; 2.3696x over previous
"""DeepSeek-style MLA transformer block on 8 Trainium2 NeuronCores.

Strategy (feature-major activations on device; weights host-pre-transposed so
every matmul chains without on-device transposes):

  Stage A (token-sharded, 256 tok/core): attn_norm -> wq_a -> q_norm and
    wkv_a -> kv_norm + rope(k_pe), all matmuls fp16 with k-slab weight DMAs
    (full-bandwidth lines) accumulating into per-m-chunk PSUM banks; the q
    and kv latents AllGather in fp16 (q first so stage B starts sooner).
  Stage B (head-sharded, 2 heads/core): wq_b (+rope q) in fp16, expand
    k_nope/v from the gathered kv latent (fp16 matmuls), exact-causal
    attention (skip fully-masked 256-col chunks; mask DMA only for the
    diagonal chunk of each query row-block, shared across heads/batch),
    fp32 softmax, probabilities cast to bf16 and transposed on the PE four
    blocks per PSUM copy, AV matmul over PAIRED query blocks (moving dim 256
    so bf16 runs 1 cyc/row); y reshards token-wise via ONE bf16 AllToAll.
  Stage C (token-sharded): full wo (bf16, prefetched during attention) on
    this core's 256 tokens + residual from the stage-A x shard; the ffn
    rmsnorm reduction is core-LOCAL (no AllReduce); res1 ships straight out
    as a kernel output and the final residual add happens on the host during
    unsharding (0.002% of the FLOPs).
  MLP (data-parallel): full SwiGLU on this core's 256 tokens in bf16,
    streaming the full w1/w3 j-tiles and w2 k-slabs (no h2 AllGather, no
    ReduceScatter); phase 2 uses g as the stationary operand so the output
    lands token-major without transposes.

Only 3 collectives total (agq, agkv, y AllToAll).  All rmsnorm weights are
folded into the following weight matrix on the host (mathematically exact);
softmax scale (-96) is folded into q at the wq_b eviction; rmsnorm
reduce+broadcast is one all-ones 128x128 fp32r matmul.  Softmax statistics
and all PSUM accumulation stay fp32.  fp16 (not bf16) is used for the
latent/QK path: the softscale (-96) amplifies latent rounding into argmax
flips, and bf16's 8-bit mantissa measurably breaches the error budget while
fp16's 10-bit mantissa keeps the final rel-err ~8e-3 (gate 2e-2).
"""

import os
import sys

sys.path.insert(0, "/opt/trn_rl_repo")

from contextlib import ExitStack

import numpy as np
import ml_dtypes

import concourse.bacc as bacc
import concourse.bass as bass
import concourse.mybir as mybir
import concourse.tile as tile
from concourse.bass_utils import run_bass_kernel_spmd
from concourse.masks import make_identity

F32 = mybir.dt.float32
F32R = mybir.dt.float32r
F16 = mybir.dt.float16
BF16 = mybir.dt.bfloat16
AX = mybir.AxisListType.X
ADD = mybir.AluOpType.add
SUB = mybir.AluOpType.subtract
MUL = mybir.AluOpType.mult
AF = mybir.ActivationFunctionType

NCORES = 8
B, T, D = 2, 1024, 2048
H = 16
NOPE, ROPE = 128, 64
QKD = NOPE + ROPE  # 192
QLORA, KVLORA = 1536, 512
VHD = 128
INTER = 8192
EPS = 1e-6
SOFTSCALE = float(QKD) * -0.5  # -96.0

N_TOK = B * T  # 2048
S = N_TOK // NCORES  # 256 tokens per core (stage A shard)
HPC = H // NCORES  # 2 heads per core
DO = D // NCORES  # 256 output-feature rows per core
ISH = INTER // NCORES  # 1024 intermediate rows per core
AGQ = QLORA
AGKV = KVLORA + ROPE  # 576

# exact-causal attention geometry: query row-block sc (128 rows) attends to
# key columns [0, 128*(sc+1)); we round the attended width up to 256 so every
# matmul's moving dim is >=256 (1 cyc/row in fp32r/bf16).  Paired row-blocks
# (2*p, 2*p+1) share the same width so their probability transposes can be
# packed side by side and the AV matmul runs with a 256-wide moving dim.
WPAIR = [256, 512, 768, 1024]
# per-sc (col_offset, width) of the single chunk that contains above-diagonal
# elements and therefore needs the additive mask; all other computed chunks
# are strictly below the diagonal and all skipped chunks are fully masked.
MASKCHUNK = [
    (0, 256), (0, 256), (0, 512), (0, 512),
    (512, 256), (512, 256), (512, 512), (512, 512),
]


def _chunks_of(w):
    """Split width w into 512-col chunks with a possible 256 tail."""
    out = []
    c = 0
    while c < w:
        step = 512 if w - c >= 512 else (w - c)
        out.append((c, step))
        c += step
    return out


_CACHED_NC = None
LAST_RESULTS = None  # test.py reads these
LAST_IN_MAPS = None


def _bc(ap):
    """View an fp32 DRAM AP as fp32r for a weight-slab DMA."""
    return ap.bitcast(F32R)


def _rms_inv(nc, ones128, eps_ap, psum_pool, work_pool, chunks, dim, n, tag):
    """chunks: list of APs [128, n] covering `dim` feature rows (feature-major).
    Returns an SBUF tile [128, n] whose every row is 1/sqrt(mean_sq + eps).
    ones128 must be an fp32r tile; the squares are written fp32r so the
    reducing matmul runs 1 cyc/row."""
    ss = psum_pool.tile([128, n], F32, tag="rms_ss", name=f"{tag}_ss")
    nchunks = len(chunks)
    for i, xc in enumerate(chunks):
        xx = work_pool.tile([128, n], F32R, tag="rms_xx", name=f"{tag}_xx")
        nc.scalar.square(xx[:], xc)
        nc.tensor.matmul(
            ss[:], ones128[:], xx[:], start=(i == 0), stop=(i == nchunks - 1)
        )
    std = work_pool.tile([128, n], F32, tag="rms_std", name=f"{tag}_std")
    nc.scalar.activation(std[:], ss[:], AF.Sqrt, bias=eps_ap, scale=1.0 / dim)
    inv = work_pool.tile([128, n], F32, tag=f"{tag}_inv", name=f"{tag}_inv")
    nc.vector.reciprocal(inv[:], std[:])
    return inv


def _rope(nc, pool, out64, in64, cos, sin, n, tag, dt=F32):
    """out64/in64: APs [64, n]; rows 0:32 = even lanes, 32:64 = odd lanes.
    cos/sin: APs [32, n] at partition base 0.  in64 may sit at any 32-aligned
    base; walrus requires equal input bases for 2-input SBUF ops, so stage the
    halves through base-0 copies first (single-input ops may shift bases)."""
    qe = pool.tile([32, n], dt, tag="rope_qe", name=f"{tag}_qe")
    qo = pool.tile([32, n], dt, tag="rope_qo", name=f"{tag}_qo")
    nc.scalar.copy(qe[:], in64[0:32, :])
    nc.scalar.copy(qo[:], in64[32:64, :])
    t0 = pool.tile([32, n], dt, tag="rope_t0", name=f"{tag}_t0")
    t1 = pool.tile([32, n], dt, tag="rope_t1", name=f"{tag}_t1")
    nc.vector.tensor_tensor(t0[:], qe[:], cos, MUL)
    nc.vector.tensor_tensor(t1[:], qo[:], sin, MUL)
    nc.vector.tensor_tensor(out64[0:32, :], t0[:], t1[:], SUB)
    t2 = pool.tile([32, n], dt, tag="rope_t2", name=f"{tag}_t2")
    t3 = pool.tile([32, n], dt, tag="rope_t3", name=f"{tag}_t3")
    nc.vector.tensor_tensor(t2[:], qe[:], sin, MUL)
    nc.vector.tensor_tensor(t3[:], qo[:], cos, MUL)
    nc.vector.tensor_tensor(out64[32:64, :], t2[:], t3[:], ADD)


def _build():
    nc = bacc.Bacc("TRN2", target_bir_lowering=False, debug=False, num_devices=NCORES)
    RG = [list(range(NCORES))]

    # ---- kernel parameters (per-core data supplied via in_maps) ----
    p_xs = nc.declare_dram_parameter("xs", [D, S], F32, isOutput=False)
    p_mask = nc.declare_dram_parameter("mask", [T, T], F32, isOutput=False)
    p_cosq = nc.declare_dram_parameter("cosq", [32, T], F16, isOutput=False)
    p_sinq = nc.declare_dram_parameter("sinq", [32, T], F16, isOutput=False)
    p_cosk = nc.declare_dram_parameter("cosk", [32, S], F32, isOutput=False)
    p_sink = nc.declare_dram_parameter("sink", [32, S], F32, isOutput=False)
    p_wqa = nc.declare_dram_parameter("wqaT", [D, QLORA], F16, isOutput=False)
    p_wqb = nc.declare_dram_parameter("wqbT", [QLORA, HPC * QKD], F16, isOutput=False)
    p_wkva = nc.declare_dram_parameter("wkvaT", [D, AGKV], F16, isOutput=False)
    p_wkb = nc.declare_dram_parameter("wkbT", [KVLORA, HPC * NOPE], F16, isOutput=False)
    p_wv = nc.declare_dram_parameter("wvT", [KVLORA, HPC * VHD], F16, isOutput=False)
    p_wo = nc.declare_dram_parameter("woT", [H * VHD, D], BF16, isOutput=False)
    p_w13 = nc.declare_dram_parameter("w13T", [D, 2 * INTER], BF16, isOutput=False)
    p_w2 = nc.declare_dram_parameter("w2T", [INTER, D], BF16, isOutput=False)
    p_mlp = nc.declare_dram_parameter("mlpT", [S, D], BF16, isOutput=True)
    p_res1 = nc.declare_dram_parameter("res1", [D, S], F32, isOutput=True)

    with tile.TileContext(
        nc, trace_sim=bool(os.environ.get("TILE_TRACE_SIM"))
    ) as tc, ExitStack() as root:
        dram = root.enter_context(tc.tile_pool(name="dram", bufs=1, space="DRAM"))
        agq_in = dram.tile([AGQ, S], F16, name="agq_in")
        agq_out = dram.tile([NCORES * AGQ, S], F16, addr_space="Shared", name="agq_out")
        agkv_in = dram.tile([AGKV, S], F16, name="agkv_in")
        agkv_out = dram.tile(
            [NCORES * AGKV, S], F16, addr_space="Shared", name="agkv_out"
        )
        y2a_in = dram.tile([H * VHD, S], BF16, name="y2a_in")
        y2a_out = dram.tile([H * VHD, S], BF16, name="y2a_out")

        const = root.enter_context(tc.tile_pool(name="const", bufs=1))
        onesf = const.tile([128, 128], F32, name="onesf")
        nc.vector.memset(onesf[:], 1.0)
        ones128 = const.tile([128, 128], F32R, name="ones128")
        nc.vector.tensor_copy(ones128[:], onesf[:])
        eps_sb = const.tile([128, 1], F32, name="eps_sb")
        nc.vector.memset(eps_sb[:], EPS)
        ident = const.tile([128, 128], BF16, name="ident")
        identf = const.tile([128, 128], F32, name="identf")
        make_identity(nc, identf[:])
        nc.vector.tensor_copy(ident[:], identf[:])
        cosq = const.tile([32, T], F16, name="cosq")
        sinq = const.tile([32, T], F16, name="sinq")

        # =========================== Stage A ===========================
        xroot = root.enter_context(tc.tile_pool(name="xroot", bufs=1))
        x_sb = xroot.tile([128, D // 128, S], F32, name="x_sb")

        wo_sb = xroot.tile([128, H * VHD // 128, D], BF16, name="wo_sb")
        with ExitStack() as sa:
            a_res = sa.enter_context(tc.tile_pool(name="a_res", bufs=1))
            a_wk = sa.enter_context(tc.tile_pool(name="a_wk", bufs=3))
            a_w = sa.enter_context(tc.tile_pool(name="a_w", bufs=3))
            a_ps2 = sa.enter_context(tc.tile_pool(name="a_ps2", bufs=2, space="PSUM"))

            for xq in range(4):
                nc.sync.dma_start(
                    x_sb[:, 4 * xq : 4 * (xq + 1), :],
                    p_xs[512 * xq : 512 * (xq + 1), :].rearrange(
                        "(ko p) n -> p ko n", p=128
                    ),
                )
            # The attn rmsnorm is a per-token scale; both the q and kv latent
            # paths re-normalize (rmsnorm) right after their linear projection,
            # so the scale cancels exactly (up to the negligible eps shift) and
            # the projections can run on RAW x.  Only the 64 decoupled-rope
            # k_pe rows keep the scale, applied after their matmul via inv_a
            # (whose rows are all equal thanks to the ones-matmul reduce).
            xh = a_res.tile([128, D // 128, S], F16, name="xh")
            for k in range(D // 128):
                nc.vector.tensor_copy(xh[:, k, :], x_sb[:, k, :])
            inv_a = _rms_inv(
                nc, ones128, eps_sb[:], a_ps2, a_wk,
                [x_sb[:, k, :] for k in range(D // 128)], D, S, "rmsa",
            )

            # q_lat = wq_a' @ xh -> [1536, S]; two m-groups of 6 accumulators
            # (group 0 is copied to SBUF so its banks free up for group 1)
            qn = QLORA // 128  # 12
            ql_parts = []
            with ExitStack() as sq:
                q_ps = sq.enter_context(
                    tc.tile_pool(name="q_ps", bufs=1, space="PSUM")
                )
                for g in range(2):
                    qps = [
                        q_ps.tile([128, S], F32, tag=f"qm{mi}", name=f"ql_ps{g}{mi}")
                        for mi in range(6)
                    ]
                    for k in range(D // 128):
                        wt = a_w.tile([128, 768], F16, tag="wqa_slab", name="wqa_t")
                        nc.sync.dma_start(
                            wt[:],
                            p_wqa[128 * k : 128 * (k + 1), 768 * g : 768 * (g + 1)],
                        )
                        for mi in range(6):
                            nc.tensor.matmul(
                                qps[mi][:], wt[:, 128 * mi : 128 * (mi + 1)],
                                xh[:, k, :],
                                start=(k == 0), stop=(k == D // 128 - 1),
                            )
                    if g == 0:
                        for mi in range(6):
                            qs = a_wk.tile(
                                [128, S], F32, tag=f"qsb{mi}", name=f"qsb{mi}"
                            )
                            nc.any.tensor_copy(qs[:], qps[mi][:])
                            ql_parts.append(qs)
                    else:
                        ql_parts.extend(qps)
                inv_q = _rms_inv(
                    nc, ones128, eps_sb[:], a_ps2, a_wk,
                    [ql_parts[m][:] for m in range(qn)], QLORA, S, "rmsq",
                )
                for m in range(qn):
                    qh = a_wk.tile([128, S], F16, tag="qh", name="qh")
                    nc.vector.tensor_tensor(qh[:], ql_parts[m][:], inv_q[:], MUL)
                    nc.sync.dma_start(agq_in[128 * m : 128 * (m + 1), :], qh[:])
            nc.gpsimd.collective_compute(
                "AllGather", mybir.AluOpType.bypass, replica_groups=RG,
                ins=[agq_in[:].opt()], outs=[agq_out[:].opt()],
            )
            # kv_all = wkv_a' @ xh -> [576, S]; rows 0:512 latent, 512:576 rope
            # k-slab weight DMAs (full-bw lines) + 5 live psum accumulators
            mchunks = [(0, 128), (128, 128), (256, 128), (384, 128), (512, 64)]
            with ExitStack() as skv:
                kv_ps = skv.enter_context(
                    tc.tile_pool(name="kv_ps", bufs=1, space="PSUM")
                )
                kvps = [
                    kv_ps.tile([128, S], F32, tag=f"kvm{mi}", name=f"kv_ps{mi}")
                    for mi in range(len(mchunks))
                ]
                for k in range(D // 128):
                    wt = a_w.tile([128, AGKV], F16, tag="wkva_slab", name="wkva_t")
                    nc.sync.dma_start(wt[:], p_wkva[128 * k : 128 * (k + 1), :])
                    for mi, (moff, msz) in enumerate(mchunks):
                        nc.tensor.matmul(
                            kvps[mi][:msz, :], wt[:, moff : moff + msz],
                            xh[:, k, :],
                            start=(k == 0), stop=(k == D // 128 - 1),
                        )
                inv_kv = _rms_inv(
                    nc, ones128, eps_sb[:], a_ps2, a_wk,
                    [kvps[m][:] for m in range(4)], KVLORA, S, "rmskv",
                )
                for m in range(4):
                    ln = a_wk.tile([128, S], F16, tag="latn", name="latn")
                    nc.vector.tensor_tensor(ln[:], kvps[m][:], inv_kv[:], MUL)
                    nc.sync.dma_start(agkv_in[128 * m : 128 * (m + 1), :], ln[:])
                # rope the decoupled key (shared across heads), feature-permuted
                # on host; computed fp32 from psum, stored fp16
                cosk = a_res.tile([32, S], F32, name="cosk")
                sink = a_res.tile([32, S], F32, name="sink")
                nc.sync.dma_start(cosk[:], p_cosk[:, :])
                nc.sync.dma_start(sink[:], p_sink[:, :])
                kpe_s = a_wk.tile([64, S], F32, tag="kpe_s", name="kpe_s")
                nc.vector.tensor_tensor(
                    kpe_s[:], kvps[4][0:64, :], inv_a[0:64, :], MUL
                )
                kpe_r = a_wk.tile([64, S], F32, tag="kpe_r", name="kpe_r")
                _rope(
                    nc, a_wk, kpe_r[:], kpe_s[:], cosk[:], sink[:], S, "ropek"
                )
                kpe_h = a_wk.tile([64, S], F16, tag="kpe_h", name="kpe_h")
                nc.scalar.copy(kpe_h[:], kpe_r[:])
                nc.sync.dma_start(
                    agkv_in[KVLORA : KVLORA + ROPE, :], kpe_h[:]
                )
            nc.gpsimd.collective_compute(
                "AllGather", mybir.AluOpType.bypass, replica_groups=RG,
                ins=[agkv_in[:].opt()], outs=[agkv_out[:].opt()],
            )

        nc.sync.dma_start(cosq[:], p_cosq[:, :])
        nc.sync.dma_start(sinq[:], p_sinq[:, :])

        # =========================== Stage B ===========================
        with ExitStack() as sb:
            b_res = sb.enter_context(tc.tile_pool(name="b_res", bufs=1))
            b_wk = sb.enter_context(tc.tile_pool(name="b_wk", bufs=2))

            k_sb = b_res.tile([128, HPC, N_TOK], F16, name="k_sb")
            kpe_sb = b_res.tile([128, N_TOK], F16, name="kpe_sb")
            v_sb = b_res.tile([128, N_TOK // 128, HPC * VHD], BF16, name="v_sb")
            q_sb = b_res.tile([128, 3, N_TOK], F16, name="q_sb")
            qpe_sb = b_res.tile([128, N_TOK], F16, name="qpe_sb")

            # ---- q for the 2 local heads (fold SOFTSCALE here) ----
            with ExitStack() as s2:
                b_w2 = s2.enter_context(tc.tile_pool(name="b_w2", bufs=1))
                b_rhs2 = s2.enter_context(tc.tile_pool(name="b_rhs2", bufs=2))
                b2_ps = s2.enter_context(
                    tc.tile_pool(name="b2_ps", bufs=2, space="PSUM")
                )
                wqb_sb = b_w2.tile([128, QLORA // 128, HPC * QKD], F16, name="wqb_sb")
                nc.sync.dma_start(
                    wqb_sb[:], p_wqb.rearrange("(ko p) m -> p ko m", p=128)
                )
                for blk in range(NCORES):
                    base = AGQ * blk
                    qrhs = b_rhs2.tile([128, QLORA // 128, S], F16, tag="qrhs", name="qrhs")
                    nc.sync.dma_start(
                        qrhs[:],
                        agq_out[base : base + QLORA, :].rearrange(
                            "(ko p) n -> p ko n", p=128
                        ),
                    )
                    psq = [
                        b2_ps.tile([128, S], F32, tag=f"psq{m}", name=f"psq{m}")
                        for m in range(3)
                    ]
                    for k in range(QLORA // 128):
                        for m in range(3):
                            nc.tensor.matmul(
                                psq[m][:], wqb_sb[:, k, 128 * m : 128 * (m + 1)],
                                qrhs[:, k, :],
                                start=(k == 0), stop=(k == QLORA // 128 - 1),
                            )
                    for m in range(3):
                        nc.scalar.mul(
                            q_sb[:, m, S * blk : S * (blk + 1)], psq[m][:], SOFTSCALE
                        )

            # ---- expand k_nope and v (token-major) for the 2 local heads ----
            with ExitStack() as s1:
                b_w1 = s1.enter_context(tc.tile_pool(name="b_w1", bufs=1))
                b_rhs1 = s1.enter_context(tc.tile_pool(name="b_rhs1", bufs=2))
                b1_ps = s1.enter_context(
                    tc.tile_pool(name="b1_ps", bufs=2, space="PSUM")
                )
                wkb_sb = b_w1.tile([128, 4, HPC * NOPE], F16, name="wkb_sb")
                nc.sync.dma_start(
                    wkb_sb[:], p_wkb.rearrange("(ko p) m -> p ko m", p=128)
                )
                wv_sb = b_w1.tile([128, 4, HPC * VHD], F16, name="wv_sb")
                nc.sync.dma_start(
                    wv_sb[:], p_wv.rearrange("(ko p) m -> p ko m", p=128)
                )
                for blk in range(NCORES):
                    base = AGKV * blk
                    lat = b_rhs1.tile([128, 4, S], F16, tag="lat", name="lat")
                    nc.sync.dma_start(
                        lat[:],
                        agkv_out[base : base + KVLORA, :].rearrange(
                            "(ko p) n -> p ko n", p=128
                        ),
                    )
                    for m in range(HPC):
                        psk = b1_ps.tile([128, S], F32, tag="psk", name="psk")
                        for k in range(4):
                            nc.tensor.matmul(
                                psk[:], wkb_sb[:, k, 128 * m : 128 * (m + 1)],
                                lat[:, k, :],
                                start=(k == 0), stop=(k == 3),
                            )
                        nc.any.tensor_copy(k_sb[:, m, S * blk : S * (blk + 1)], psk[:])
                    for th in range(S // 128):
                        psv = b1_ps.tile([128, HPC * VHD], F32, tag="psv", name="psv")
                        for k in range(4):
                            nc.tensor.matmul(
                                psv[:], lat[:, k, 128 * th : 128 * (th + 1)],
                                wv_sb[:, k, :], start=(k == 0), stop=(k == 3),
                            )
                        nc.any.tensor_copy(v_sb[:, (S // 128) * blk + th, :], psv[:])
                    nc.sync.dma_start(
                        kpe_sb[0:64, S * blk : S * (blk + 1)],
                        agkv_out[base + KVLORA : base + KVLORA + ROPE, :],
                    )
                    nc.sync.dma_start(
                        kpe_sb[64:128, S * blk : S * (blk + 1)],
                        agkv_out[base + KVLORA : base + KVLORA + ROPE, :],
                    )

            # rope q_pe: q_sb chunk 2 = [h0_e, h0_o, h1_e, h1_o] x32 rows
            with tc.tile_pool(name="rope_wk", bufs=1) as rp:
                for b in range(B):
                    for h in range(HPC):
                        _rope(
                            nc, rp,
                            qpe_sb[64 * h : 64 * (h + 1), T * b : T * (b + 1)],
                            q_sb[64 * h : 64 * (h + 1), 2, T * b : T * (b + 1)],
                            cosq[:, :], sinq[:, :], T, f"ropeq{b}{h}", dt=F16,
                        )

            # prefetch the full wo (consumed at stage C) during attention, in
            # 16 chunks so stage A/B loads can interleave instead of
            # head-blocking behind one 25us transfer
            for k in range(H * VHD // 128):
                nc.sync.dma_start(wo_sb[:, k, :], p_wo[128 * k : 128 * (k + 1), :])

            # ---- mask: one diagonal chunk per query row-block, shared h/b ----
            mtile = b_res.tile([128, T // 128, 512], F32, name="mtile")
            for sc in range(T // 128):
                c0, w = MASKCHUNK[sc]
                nc.sync.dma_start(
                    mtile[:, sc, 0:w],
                    p_mask[128 * sc : 128 * (sc + 1), c0 : c0 + w],
                )

            # ---- attention: exact-causal, paired row-blocks for 256-wide AV ----
            with ExitStack() as s3:
                b3_ps = s3.enter_context(
                    tc.tile_pool(name="b3_ps", bufs=2, space="PSUM")
                )
                for h in range(HPC):
                    for pair in range(T // 256):
                        W = WPAIR[pair]
                        ntj = W // 128
                        for b in range(B):
                            ptrp = b_wk.tile(
                                [128, T // 128, 256], BF16, tag="ptrp", name="ptrp"
                            )
                            for blk2 in range(2):
                                sc = 2 * pair + blk2
                                s0 = T * b + 128 * sc
                                scs = b_wk.tile([128, T], F32, tag="scs", name="scs")
                                for (c0, w) in _chunks_of(W):
                                    t0 = T * b + c0
                                    ps = b3_ps.tile(
                                        [128, 512], F32, tag="ps_qk", name="ps_qk"
                                    )
                                    nc.tensor.matmul(
                                        ps[:, :w],
                                        q_sb[:, h, s0 : s0 + 128],
                                        k_sb[:, h, t0 : t0 + w],
                                        start=True, stop=False,
                                    )
                                    nc.tensor.matmul(
                                        ps[:, :w],
                                        qpe_sb[64 * h : 64 * (h + 1), s0 : s0 + 128],
                                        kpe_sb[64 * h : 64 * (h + 1), t0 : t0 + w],
                                        start=False, stop=True,
                                    )
                                    if (c0, w) == MASKCHUNK[sc]:
                                        nc.vector.tensor_tensor(
                                            scs[:, c0 : c0 + w], ps[:, :w],
                                            mtile[:, sc, 0:w], ADD,
                                        )
                                    else:
                                        nc.any.tensor_copy(
                                            scs[:, c0 : c0 + w], ps[:, :w]
                                        )
                                nmax = b_wk.tile([128, 1], F32, tag="nmax", name="nmax")
                                nc.vector.reduce_max(
                                    nmax[:], scs[:, :W], axis=AX, negate=True
                                )
                                p_sb = b_wk.tile([128, T], F32, tag="p_sb", name="p_sb")
                                zsum = b_wk.tile([128, 1], F32, tag="zsum", name="zsum")
                                nc.scalar.activation(
                                    p_sb[:, :W], scs[:, :W], AF.Exp,
                                    bias=nmax[:], accum_out=zsum[:],
                                )
                                invz = b_wk.tile([128, 1], F32, tag="invz", name="invz")
                                nc.vector.reciprocal(invz[:], zsum[:])
                                pn = b_wk.tile([128, T], BF16, tag="pn", name="pn")
                                nc.vector.tensor_scalar_mul(
                                    pn[:, :W], p_sb[:, :W], invz[:]
                                )
                                for tj0 in range(0, ntj, 4):
                                    tjn = min(4, ntj - tj0)
                                    pst = b3_ps.tile(
                                        [128, 4, 128], BF16, tag="pst", name="pst"
                                    )
                                    for i in range(tjn):
                                        nc.tensor.transpose(
                                            pst[:, i, :],
                                            pn[:, 128 * (tj0 + i) : 128 * (tj0 + i + 1)],
                                            ident[:],
                                        )
                                    nc.any.tensor_copy(
                                        ptrp[
                                            :, tj0 : tj0 + tjn,
                                            128 * blk2 : 128 * (blk2 + 1),
                                        ],
                                        pst[:, 0:tjn, :],
                                    )
                            psy = b3_ps.tile([128, 256], F32, tag="psy", name="psy")
                            for tj in range(ntj):
                                nc.tensor.matmul(
                                    psy[:],
                                    v_sb[:, (T // 128) * b + tj, VHD * h : VHD * (h + 1)],
                                    ptrp[:, tj, :],
                                    start=(tj == 0), stop=(tj == ntj - 1),
                                )
                            yst = b_wk.tile([128, 256], BF16, tag="yst", name="yst")
                            nc.any.tensor_copy(yst[:], psy[:])
                            d = 4 * b + pair
                            nc.sync.dma_start(
                                y2a_in[256 * d + 128 * h : 256 * d + 128 * (h + 1), :],
                                yst[:],
                            )

            # AllToAll chunk d (rows [256d, 256d+256)) went to core d: my 2
            # heads x d's 256 tokens (written per-block during attention).
            # Received chunk e = e's heads 2e,2e+1 for MY tokens -> y2a_out is
            # exactly [all 16 heads' vhd, my tokens].
            nc.gpsimd.collective_compute(
                "AllToAll", mybir.AluOpType.bypass, replica_groups=RG,
                ins=[y2a_in[:].opt()], outs=[y2a_out[:].opt()],
            )

        # ====== Stage C: token-sharded wo + LOCAL ffn-norm (no AllReduce) ======
        # Each core computes attn-out/res1/h2 for its OWN 256 tokens over the
        # full feature dim: the ffn rmsnorm reduction is core-local, h2 feeds a
        # DATA-PARALLEL MLP directly (no h2 AllGather, no ReduceScatter), and
        # res1 ships straight out as a kernel output (host does the final add).
        h2root = root.enter_context(tc.tile_pool(name="h2root", bufs=1))
        h2n = h2root.tile([128, D // 128, S], BF16, name="h2n")
        binv = h2root.tile([128, S], F32, name="binv")
        with ExitStack() as sc_stack:
            c_res = sc_stack.enter_context(tc.tile_pool(name="c_res", bufs=1))
            c_wk = sc_stack.enter_context(tc.tile_pool(name="c_wk", bufs=3))
            c_ps = sc_stack.enter_context(
                tc.tile_pool(name="c_ps", bufs=2, space="PSUM")
            )

            yloc = c_res.tile([128, H * VHD // 128, S], BF16, name="yloc")
            for yq in range(4):
                nc.sync.dma_start(
                    yloc[:, 4 * yq : 4 * (yq + 1), :],
                    y2a_out[512 * yq : 512 * (yq + 1), :].rearrange(
                        "(ko p) n -> p ko n", p=128
                    ),
                )
            res1f = c_res.tile([128, D // 128, S], F32, name="res1f")
            for dm in range(D // 128):
                psD = c_ps.tile([128, S], F32, tag="psD", name="psD")
                for k in range(H * VHD // 128):
                    nc.tensor.matmul(
                        psD[:], wo_sb[:, k, 128 * dm : 128 * (dm + 1)], yloc[:, k, :],
                        start=(k == 0), stop=(k == H * VHD // 128 - 1),
                    )
                nc.vector.tensor_tensor(res1f[:, dm, :], psD[:], x_sb[:, dm, :], ADD)

            # h2n is the RAW residual in bf16: the ffn-norm scale commutes
            # past the linear w1/w3 matmuls (silu((w1 r)*inv) = silu(w1 (r*inv)))
            # and is applied per-token at the silu step via binv, so the MLP
            # can start the moment res1f chunks exist while the rms reduction
            # runs concurrently.
            for k in range(D // 128):
                nc.vector.tensor_copy(h2n[:, k, :], res1f[:, k, :])
            inv_f = _rms_inv(
                nc, ones128, eps_sb[:], c_ps, c_wk,
                [res1f[:, k, :] for k in range(D // 128)], D, S, "rmsf",
            )
            nc.vector.tensor_copy(binv[:], inv_f[:])
            # res1 goes straight out as a kernel output; the final residual
            # add happens on the host during unsharding (0.002% of the FLOPs).
            nc.sync.dma_start(p_res1.rearrange("(ko p) n -> p ko n", p=128), res1f[:])

        # ---- DATA-PARALLEL MLP: full SwiGLU on this core's 256 tokens ----
        # Streams the full w1/w3 (j-tiles) and w2 (k-slabs) in bf16; all
        # matmuls have a 256-wide moving dim (the token count).  Phase 2
        # computes the output TOKEN-major (tokens on partitions) by using g as
        # the stationary operand, so the mlpT output needs no transposes.
        NJ = 2 * INTER // 256  # 64 w1/w3 j-tiles
        NK = INTER // 128  # 64 w2 k-slabs
        with ExitStack() as sm:
            m_res = sm.enter_context(tc.tile_pool(name="m_res", bufs=1))
            m_w = sm.enter_context(tc.tile_pool(name="m_w", bufs=3))
            m_wk = sm.enter_context(tc.tile_pool(name="m_wk", bufs=3))
            g_sb = m_res.tile([128, NJ, S], BF16, name="g_sb")
            with ExitStack() as sm1:
                m_ps1 = sm1.enter_context(
                    tc.tile_pool(name="m_ps1", bufs=2, space="PSUM")
                )
                for j in range(NJ):
                    wj = m_w.tile([128, D // 128, 256], BF16, tag="wj", name="wj")
                    nc.sync.dma_start(
                        wj[:],
                        p_w13[:, 256 * j : 256 * (j + 1)].rearrange(
                            "(ko p) m -> p ko m", p=128
                        ),
                    )
                    psa = m_ps1.tile([128, S], F32, tag="psa", name="psa")
                    psb = m_ps1.tile([128, S], F32, tag="psb", name="psb")
                    for k in range(D // 128):
                        nc.tensor.matmul(
                            psa[:], wj[:, k, 0:128], h2n[:, k, :],
                            start=(k == 0), stop=(k == D // 128 - 1),
                        )
                        nc.tensor.matmul(
                            psb[:], wj[:, k, 128:256], h2n[:, k, :],
                            start=(k == 0), stop=(k == D // 128 - 1),
                        )
                    sa = m_wk.tile([128, S], F32, tag="sa", name="sa")
                    nc.vector.tensor_tensor(sa[:], psa[:], binv[:], MUL)
                    sb_ = m_wk.tile([128, S], F32, tag="sb_", name="sb_")
                    nc.vector.tensor_tensor(sb_[:], psb[:], binv[:], MUL)
                    tsi = m_wk.tile([128, S], F32, tag="tsi", name="tsi")
                    nc.scalar.activation(tsi[:], sa[:], AF.Silu)
                    nc.vector.tensor_tensor(g_sb[:, j, :], tsi[:], sb_[:], MUL)
            with ExitStack() as sm2:
                m_ps2 = sm2.enter_context(
                    tc.tile_pool(name="m_ps2", bufs=1, space="PSUM")
                )
                pso = [
                    m_ps2.tile([128, 512], F32, tag=f"pso{tb}_{dc}", name=f"pso{tb}{dc}")
                    for tb in range(S // 128)
                    for dc in range(D // 512)
                ]
                for k in range(NK):
                    w2k = m_w.tile([128, D], BF16, tag="w2k", name="w2k")
                    nc.sync.dma_start(w2k[:], p_w2[128 * k : 128 * (k + 1), :])
                    for tb in range(S // 128):
                        for dc in range(D // 512):
                            nc.tensor.matmul(
                                pso[tb * (D // 512) + dc][:],
                                g_sb[:, k, 128 * tb : 128 * (tb + 1)],
                                w2k[:, 512 * dc : 512 * (dc + 1)],
                                start=(k == 0), stop=(k == NK - 1),
                            )
                for tb in range(S // 128):
                    for dc in range(D // 512):
                        o_sb = m_wk.tile([128, 512], BF16, tag="o_sb", name="o_sb")
                        nc.any.tensor_copy(o_sb[:], pso[tb * (D // 512) + dc][:])
                        nc.sync.dma_start(
                            p_mlp[
                                128 * tb : 128 * (tb + 1), 512 * dc : 512 * (dc + 1)
                            ],
                            o_sb[:],
                        )

    nc.compile()
    return nc


def _rope_perm(n):
    """Permutation putting even lanes first then odd lanes, for an n-row rope
    block (n even): [0,2,4,...,n-2, 1,3,5,...,n-1]."""
    return np.concatenate([np.arange(0, n, 2), np.arange(1, n, 2)])


def kernel(**inputs):
    global _CACHED_NC, LAST_RESULTS, LAST_IN_MAPS
    f32 = lambda a: np.ascontiguousarray(np.asarray(a), dtype=np.float32)
    f16 = lambda a: np.ascontiguousarray(np.asarray(a), dtype=np.float16)
    bf16 = lambda a: np.ascontiguousarray(np.asarray(a), dtype=ml_dtypes.bfloat16)

    x = f32(inputs["x"]).reshape(N_TOK, D)
    mask = f32(inputs["mask"])
    cos = f32(inputs["freqs_cos"])  # [T, 32]
    sin = f32(inputs["freqs_sin"])
    attn_nw = f32(inputs["attn_norm_w"])
    wq_a = f32(inputs["wq_a"]) * attn_nw[None, :]
    q_nw = f32(inputs["q_norm_w"])
    wq_b = f32(inputs["wq_b"]) * q_nw[None, :]
    wkv_a = f32(inputs["wkv_a"]) * attn_nw[None, :]
    kv_nw = f32(inputs["kv_norm_w"])
    wkv_b = f32(inputs["wkv_b"]) * kv_nw[None, :]
    wo = f32(inputs["wo"])
    ffn_nw = f32(inputs["ffn_norm_w"])
    w1 = f32(inputs["w1"]) * ffn_nw[None, :]
    w3 = f32(inputs["w3"]) * ffn_nw[None, :]
    w2 = f32(inputs["w2"])

    xT = np.ascontiguousarray(x.T)  # [D, N_TOK] feature-major
    cosT = np.ascontiguousarray(cos.T)  # [32, T]
    sinT = np.ascontiguousarray(sin.T)

    # wkv_a rows: keep 0:512 (latent); permute rope rows 512:576 to even|odd
    pk = _rope_perm(ROPE)
    wkva_p = wkv_a.copy()
    wkva_p[KVLORA:] = wkv_a[KVLORA:][pk]
    wkvaT = np.ascontiguousarray(wkva_p.T)  # [D, 576]

    wqaT = np.ascontiguousarray(wq_a.T)  # [D, 1536]
    # full wo, transposed, natural hv row order (AllToAll lands head k's y at
    # row block k); shared across cores
    woT = bf16(wo.T)  # [2048, 2048]
    # full w1/w3 interleaved in 256-col j-tiles (128 w1 | 128 w3), and full
    # w2 transposed; shared across cores (data-parallel MLP)
    w13T = np.empty((D, 2 * INTER), np.float32)
    for j in range(INTER // 128):
        w13T[:, 256 * j : 256 * j + 128] = w1[128 * j : 128 * (j + 1)].T
        w13T[:, 256 * j + 128 : 256 * (j + 1)] = w3[128 * j : 128 * (j + 1)].T
    w13T = bf16(w13T)
    w2T = bf16(w2.T)  # [8192, 2048]

    in_maps = []
    for c in range(NCORES):
        heads = [HPC * c + j for j in range(HPC)]
        # wq_b rows per head h: h*QKD .. h*QKD+192 (128 nope + 64 rope)
        # target col order: [h0_nope(128), h1_nope(128), h0_rope_eo(64), h1_rope_eo(64)]
        cols = []
        for h in heads:
            cols.append(wq_b[h * QKD : h * QKD + NOPE])
        for h in heads:
            cols.append(wq_b[h * QKD + NOPE : (h + 1) * QKD][pk])
        wqbT = f16(np.concatenate(cols, axis=0).T)  # [1536, 384]

        # wkv_b rows per head h: h*(NOPE+VHD) + [0:128]=k_nope, [128:256]=v
        kw = np.concatenate(
            [wkv_b[h * (NOPE + VHD) : h * (NOPE + VHD) + NOPE] for h in heads], axis=0
        )
        vw = np.concatenate(
            [wkv_b[h * (NOPE + VHD) + NOPE : (h + 1) * (NOPE + VHD)] for h in heads],
            axis=0,
        )
        wkbT = f16(kw.T)  # [512, 256]
        wvT = f16(vw.T)  # [512, 256]


        tpos = (S * c) % T  # position within batch of this token shard
        in_maps.append(
            {
                "xs": np.ascontiguousarray(xT[:, S * c : S * (c + 1)]),
                "mask": mask,
                "cosq": f16(cosT),
                "sinq": f16(sinT),
                "cosk": np.ascontiguousarray(cosT[:, tpos : tpos + S]),
                "sink": np.ascontiguousarray(sinT[:, tpos : tpos + S]),
                "wqaT": f16(wqaT),
                "wqbT": wqbT,
                "wkvaT": f16(wkvaT),
                "wkbT": wkbT,
                "wvT": wvT,
                "woT": woT,
                "w13T": w13T,
                "w2T": w2T,
            }
        )

    LAST_IN_MAPS = in_maps
    if _CACHED_NC is None:
        _CACHED_NC = _build()
    nc = _CACHED_NC

    trace = bool(os.environ.get("KERNEL_TRACE"))
    res = run_bass_kernel_spmd(
        nc, in_maps, core_ids=list(range(NCORES)), trace=trace
    )
    LAST_RESULTS = res

    # final residual add during unsharding: both mlpT [S, D] (token-major)
    # and res1 [D, S] (feature-major) are token-sharded per core
    out = np.concatenate(
        [
            np.asarray(res.results[c]["mlpT"]).astype(np.float32)
            + np.asarray(res.results[c]["res1"]).T
            for c in range(NCORES)
        ],
        axis=0,
    )
    return np.ascontiguousarray(out).reshape(B, T, D).astype(np.float32)
